# revision 1
# baseline (speedup 1.0000x reference)
"""Trainium2 Bass kernel for nn_ConvSPE (depthwise-conv SPE + per-channel contraction).

Math (reference): per bn=(b,nu) row and channel d:
    pe_k = noise / sqrt(num*d)                       (b*num, d, s+2k)
    pe_q = depthwise_valid_xcorr(pe_k, w)            k=200 taps, same filter per channel
    qhat[b,nu,t] = sum_d pe_q[bn,d,t]      * q[b,d,t]
    khat[b,nu,t] = sum_d pe_k[bn,d,t+k//2] * k[b,d,t]

Kernel strategy (8 NeuronCores, data-parallel over the 128 bn rows; 16 rows/core):
  * Host pre-arranges noise into a time-partition-inner fp16 layout
    xf[bn][p, n, d] = noise[bn, d, 128n+p] so the conv becomes 3 PSUM-accumulated
    TensorE matmuls per output block with fixed Toeplitz weights
    W_s[p, m] = w[p + 128s - m] (shared across all channels/rows).
  * qhat: ScalarE copies conv PSUM -> SBUF fp16, VectorE multiplies by
    host-pre-transposed queries (fp16 2x mode) and reduces over d.
  * khat needs no conv: VectorE multiplies xf by a host-shifted/scaled keys
    layout (shift k//2=100 and 1/sqrt(num*d) baked in); GpSimd reduces over d
    with an fp32 add-tree (engine balancing: DVE is the bottleneck).
"""

import math
import numpy as np

_CACHE = {}


def _ensure_paths():
    try:
        import concourse  # noqa: F401
    except ImportError:
        import sys

        for p in ("/opt/trn_rl_repo", "/root/.axon_site/_ro/trn_rl_repo"):
            if p not in sys.path:
                sys.path.insert(0, p)


N_CORES = 8
B, D, L, K, NUM = 4, 64, 4096, 200, 32
NW = 34  # x windows of 128 loaded per row (covers t+j up to 4351)
NT = 32  # output time blocks of 128
NK = 33  # khat product blocks (u = t + 100 spans [0, 4224))
ROWS = 16  # bn rows per core


def _add_tree(eng, pool, acc_out, src, n_outer, bn, mybir):
    """Reduce src [128, n_outer, 64] over the last axis into acc_out [128, n_outer]
    using TT-adds (fp32 after level 1). Works on engines without X-axis reduce."""
    F32 = mybir.dt.float32
    a = pool.tile([128, n_outer, 32], F32, tag="treeA", name=f"treeA_{bn}")
    b = pool.tile([128, n_outer, 16], F32, tag="treeB", name=f"treeB_{bn}")
    eng.tensor_add(a[:], src[:, :, 0:32], src[:, :, 32:64])
    eng.tensor_add(b[:], a[:, :, 0:16], a[:, :, 16:32])
    eng.tensor_add(a[:, :, 0:8], b[:, :, 0:8], b[:, :, 8:16])
    eng.tensor_add(b[:, :, 0:4], a[:, :, 0:4], a[:, :, 4:8])
    eng.tensor_add(a[:, :, 8:10], b[:, :, 0:2], b[:, :, 2:4])
    eng.tensor_add(acc_out, a[:, :, 8], a[:, :, 9])


def build_module():
    """Build + compile the per-core Bass module (identical SPMD program)."""
    _ensure_paths()
    from contextlib import ExitStack

    import concourse.bacc as bacc
    import concourse.mybir as mybir
    import concourse.tile as tile

    F16 = mybir.dt.float16
    F32 = mybir.dt.float32
    X = mybir.AxisListType.X

    nc = bacc.Bacc(
        "TRN2", target_bir_lowering=False, debug=False, num_devices=N_CORES
    )

    xf_d = nc.dram_tensor("xf", [ROWS, 128, NW, D], F16, kind="ExternalInput").ap()
    wq_d = nc.dram_tensor("wq", [3, 128, 128], F16, kind="ExternalInput").ap()
    qt_d = nc.dram_tensor("qt", [128, NT, D], F16, kind="ExternalInput").ap()
    kf_d = nc.dram_tensor("kf", [128, NK, D], F16, kind="ExternalInput").ap()
    qo_d = nc.dram_tensor("qo", [128, ROWS, NT], F32, kind="ExternalOutput").ap()
    ko_d = nc.dram_tensor("ko", [128, ROWS, NK], F32, kind="ExternalOutput").ap()

    with tile.TileContext(nc) as tc, ExitStack() as ctx:
        wp = ctx.enter_context(tc.tile_pool(name="const", bufs=1))
        xp = ctx.enter_context(tc.tile_pool(name="x", bufs=4))
        pp = ctx.enter_context(tc.tile_pool(name="psum", bufs=4, space="PSUM"))
        cp = ctx.enter_context(tc.tile_pool(name="peq", bufs=3))
        qp = ctx.enter_context(tc.tile_pool(name="prodq", bufs=3))
        kpool = ctx.enter_context(tc.tile_pool(name="prodk", bufs=3))
        tp = ctx.enter_context(tc.tile_pool(name="tree", bufs=3))
        op = ctx.enter_context(tc.tile_pool(name="out", bufs=1))

        wts = []
        for s in range(3):
            t = wp.tile([128, 128], F16, tag=f"w{s}")
            nc.sync.dma_start(t[:], wq_d[s])
            wts.append(t)
        qt_t = wp.tile([128, NT, D], F16, tag="qt")
        nc.sync.dma_start(qt_t[:], qt_d[:])
        kf_t = wp.tile([128, NK, D], F16, tag="kf")
        nc.sync.dma_start(kf_t[:], kf_d[:])

        qacc = op.tile([128, ROWS, NT], F32, tag="qa")
        kacc = op.tile([128, ROWS, NK], F32, tag="ka")

        for bn in range(ROWS):
            xt = xp.tile([128, NW, D], F16, tag="xt", name=f"xt_{bn}")
            nc.sync.dma_start(xt[:], xf_d[bn])

            # ---- khat path: pure elementwise + gpsimd reduce tree
            pk = kpool.tile([128, NK, D], F16, tag="pk", name=f"pk_{bn}")
            nc.vector.tensor_mul(pk[:], xt[:, 0:NK, :], kf_t[:])
            _add_tree(nc.gpsimd, tp, kacc[:, bn, :], pk, NK, bn, mybir)

            # ---- qhat path: conv via 3 Toeplitz matmuls per 8-block group.
            # Two 2-bank PSUM halves per row so ACT/DVE drain half 0 while
            # PE still works on half 1.
            for h in range(2):
                ps = pp.tile([128, NT // 2, D], F32, tag="ps", name=f"ps_{bn}_{h}")
                for s in range(3):
                    for g in range(2 * h, 2 * h + 2):
                        nc.tensor.matmul(
                            ps[:, (g - 2 * h) * 8 : (g - 2 * h + 1) * 8, :],
                            wts[s][:],
                            xt[:, g * 8 + s : g * 8 + s + 8, :],
                            start=(s == 0),
                            stop=(s == 2),
                        )
                peq = cp.tile([128, NT // 2, D], F16, tag="peq", name=f"peq_{bn}_{h}")
                nc.scalar.copy(peq[:], ps[:])
                pq = qp.tile([128, NT // 2, D], F16, tag="pq", name=f"pq_{bn}_{h}")
                nc.vector.tensor_mul(
                    pq[:], peq[:], qt_t[:, h * (NT // 2) : (h + 1) * (NT // 2), :]
                )
                nc.vector.reduce_sum(
                    qacc[:, bn, h * (NT // 2) : (h + 1) * (NT // 2)], pq[:], axis=X
                )

        nc.sync.dma_start(qo_d[:], qacc[:])
        nc.sync.dma_start(ko_d[:], kacc[:])

    nc.compile()
    return nc


def _get_module():
    if "nc" not in _CACHE:
        _CACHE["nc"] = build_module()
    return _CACHE["nc"]


def make_in_maps(queries, keys, noise, conv_weight, num):
    """Host-side shard + re-layout (all cheap numpy ops)."""
    num = int(np.asarray(num))
    queries = np.asarray(queries, dtype=np.float32)
    keys = np.asarray(keys, dtype=np.float32)
    noise = np.asarray(noise, dtype=np.float32)
    w = np.asarray(conv_weight, dtype=np.float32)[0, 0, :]
    scale = 1.0 / math.sqrt(num * D)

    # Toeplitz weights (scale folded in): W_s[p, m] = w[p + 128s - m] * scale
    p = np.arange(128)[:, None]
    m = np.arange(128)[None, :]
    Wq = np.zeros((3, 128, 128), np.float32)
    for s in range(3):
        j = p + 128 * s - m
        mask = (j >= 0) & (j < K)
        Wq[s][mask] = w[j[mask]] * scale
    Wq16 = Wq.astype(np.float16)

    # xf[bn][p, n, d] = noise[bn, d, 128n + p]
    xf = (
        noise[:, :, : NW * 128]
        .reshape(B * NUM, D, NW, 128)
        .transpose(0, 3, 2, 1)
        .astype(np.float16)
    )
    # qt[b][p, tau, d] = queries[b, d, 128 tau + p]
    qt = queries.reshape(B, D, NT, 128).transpose(0, 3, 2, 1).astype(np.float16)
    # kf[b][p, n, d] = keys[b, d, 128n + p - 100] * scale (zero out of range)
    kp = np.zeros((B, D, NK * 128), np.float32)
    kp[:, :, K // 2 : K // 2 + L] = keys * scale
    kf = kp.reshape(B, D, NK, 128).transpose(0, 3, 2, 1).astype(np.float16)

    in_maps = []
    for c in range(N_CORES):
        b = c // 2
        in_maps.append(
            {
                "xf": np.ascontiguousarray(xf[ROWS * c : ROWS * (c + 1)]),
                "wq": Wq16,
                "qt": np.ascontiguousarray(qt[b]),
                "kf": np.ascontiguousarray(kf[b]),
            }
        )
    return in_maps


def assemble_outputs(results):
    qhat = np.empty((B * NUM, L), np.float32)
    khat = np.empty((B * NUM, L), np.float32)
    for c in range(N_CORES):
        qo = results[c]["qo"]  # [128, ROWS, NT]
        ko = results[c]["ko"]  # [128, ROWS, NK]
        qhat[ROWS * c : ROWS * (c + 1)] = qo.transpose(1, 2, 0).reshape(ROWS, L)
        kv = ko.transpose(1, 2, 0).reshape(ROWS, NK * 128)
        khat[ROWS * c : ROWS * (c + 1)] = kv[:, K // 2 : K // 2 + L]
    return (
        qhat.reshape(B, NUM, L),
        khat.reshape(B, NUM, L),
    )


def kernel(queries, keys, noise, conv_weight, num):
    _ensure_paths()
    from concourse import bass_utils

    in_maps = make_in_maps(queries, keys, noise, conv_weight, num)
    nc = _get_module()
    res = bass_utils.run_bass_kernel_spmd(nc, in_maps, core_ids=list(range(N_CORES)))
    return assemble_outputs(res.results)



# revision 3
# speedup vs baseline: 1.3692x; 1.3692x over previous
"""Trainium2 Bass kernel for nn_ConvSPE (depthwise-conv SPE + per-channel contraction).

Math (reference): per bn=(b,nu) row and channel d:
    pe_k = noise / sqrt(num*d)                       (b*num, d, s+2k)
    pe_q = depthwise_valid_xcorr(pe_k, w)            k=200 taps, same filter per channel
    qhat[b,nu,t] = sum_d pe_q[bn,d,t]      * q[b,d,t]
    khat[b,nu,t] = sum_d pe_k[bn,d,t+k//2] * k[b,d,t]

Kernel strategy (8 NeuronCores, data-parallel over 128 bn rows; 8 row-PAIRS/core):
  * Transposed-conv orientation: stationary = x-window [sample, (r,d)],
    moving = Toeplitz W_s[sample, t'] -> PSUM holds pe_q^T [(r,d), t'].
    3 PSUM-accumulated matmuls per (pair, t-block), 128 cols each.
  * q-path: ACT drains PSUM->SBUF fp16; DVE multiplies by replicated q^T
    (fp16 2x); the d-reduction is a PE matmul with the products as the
    STATIONARY and a [128,2] row-selector as moving -> cost 2 cols.
  * k-path: DVE mul vs shifted/scaled keys (fp16 2x); reduce over d via a
    split tree: Pool takes L1 + tail reduce, DVE takes L2+L3.
"""

import math
import numpy as np

_CACHE = {}


def _ensure_paths():
    try:
        import concourse  # noqa: F401
    except ImportError:
        import sys

        for p in ("/opt/trn_rl_repo", "/root/.axon_site/_ro/trn_rl_repo"):
            if p not in sys.path:
                sys.path.insert(0, p)


N_CORES = 8
B, D, L, K, NUM = 4, 64, 4096, 200, 32
NW = 34  # x windows of 128 per pair tile (covers t+j up to 4351)
NT = 32  # output time blocks of 128
NK = 33  # khat product blocks (u = t + 100 spans [0, 4224))
PAIRS = 8  # row-pairs per core (16 rows)


def build_module():
    """Build + compile the per-core Bass module (identical SPMD program)."""
    _ensure_paths()
    from contextlib import ExitStack

    import concourse.bacc as bacc
    import concourse.mybir as mybir
    import concourse.tile as tile

    F16 = mybir.dt.float16
    F32 = mybir.dt.float32
    X = mybir.AxisListType.X

    nc = bacc.Bacc(
        "TRN2", target_bir_lowering=False, debug=False, num_devices=N_CORES
    )

    xf_d = nc.dram_tensor("xf", [PAIRS, 128, NW, 128], F16, kind="ExternalInput").ap()
    wq_d = nc.dram_tensor("wq", [3, 128, 128], F16, kind="ExternalInput").ap()
    qt_d = nc.dram_tensor("qt", [128, NT, 128], F16, kind="ExternalInput").ap()
    kt_d = nc.dram_tensor("kt", [128, NK, D], F16, kind="ExternalInput").ap()
    sel_d = nc.dram_tensor("sel", [128, 2], F16, kind="ExternalInput").ap()
    qo_d = nc.dram_tensor("qo", [PAIRS, 128, NT, 2], F32, kind="ExternalOutput").ap()
    ko_d = nc.dram_tensor("ko", [PAIRS, 128, NK, 2], F16, kind="ExternalOutput").ap()

    with tile.TileContext(nc) as tc, ExitStack() as ctx:
        wp = ctx.enter_context(tc.tile_pool(name="const", bufs=1))
        xp = ctx.enter_context(tc.tile_pool(name="x", bufs=3))
        pp = ctx.enter_context(tc.tile_pool(name="psum", bufs=2, space="PSUM"))
        op_ps = ctx.enter_context(tc.tile_pool(name="opsum", bufs=2, space="PSUM"))
        cp = ctx.enter_context(tc.tile_pool(name="peq", bufs=2))
        qp = ctx.enter_context(tc.tile_pool(name="prodq", bufs=2))
        kp = ctx.enter_context(tc.tile_pool(name="prodk", bufs=2))
        tp = ctx.enter_context(tc.tile_pool(name="tree", bufs=2))
        oq = ctx.enter_context(tc.tile_pool(name="outq", bufs=2))
        ok = ctx.enter_context(tc.tile_pool(name="outk", bufs=2))

        # consts: Toeplitz weights first (small, gates conv start)
        wts = []
        for s in range(3):
            t = wp.tile([128, 128], F16, tag=f"w{s}")
            nc.sync.dma_start(t[:], wq_d[s])
            wts.append(t)
        sel_t = wp.tile([128, 2], F16, tag="sel")
        nc.sync.dma_start(sel_t[:], sel_d[:])

        # first x tile before the bigger consts so conv starts early
        xts = {}
        xts[0] = xp.tile([128, NW, 128], F16, tag="xt", name="xt_0")
        nc.sync.dma_start(xts[0][:], xf_d[0])

        kt_t = wp.tile([128, NK, D], F16, tag="kt")
        nc.sync.dma_start(kt_t[:], kt_d[:])
        qt_t = wp.tile([128, NT, 128], F16, tag="qt")
        nc.sync.dma_start(qt_t[:], qt_d[:])

        pq_tiles = {}

        def emit_qreduce(pr):
            """PE d-reduce for pair pr (products ready), then drain + DMA out."""
            pq = pq_tiles.pop(pr)
            po = op_ps.tile([128, NT, 2], F32, tag="po", name=f"po_{pr}")
            for t in range(NT):
                nc.tensor.matmul(
                    po[:, t, :], pq[:, t, :], sel_t[:], start=True, stop=True
                )
            qo_s = oq.tile([128, NT, 2], F32, tag="qo", name=f"qo_{pr}")
            nc.scalar.copy(qo_s[:], po[:])
            nc.sync.dma_start(qo_d[pr], qo_s[:])

        for pr in range(PAIRS):
            xt = xts.pop(pr)
            if pr + 1 < PAIRS:
                xts[pr + 1] = xp.tile([128, NW, 128], F16, tag="xt", name=f"xt_{pr+1}")
                nc.sync.dma_start(xts[pr + 1][:], xf_d[pr + 1])

            # ---- conv: 4 chunks of 8 t-blocks; PSUM [(r,d), t']
            peq = cp.tile([128, NT, 128], F16, tag="peq", name=f"peq_{pr}")
            for ch in range(4):
                ps = pp.tile([128, 8, 128], F32, tag="ps", name=f"ps_{pr}_{ch}")
                for t8 in range(8):
                    tau = ch * 8 + t8
                    for s in range(3):
                        nc.tensor.matmul(
                            ps[:, t8, :],
                            xt[:, tau + s, :],
                            wts[s][:],
                            start=(s == 0),
                            stop=(s == 2),
                        )
                nc.scalar.copy(peq[:, ch * 8 : (ch + 1) * 8, :], ps[:])
                # software-pipeline: previous pair's q-reduce between chunks
                if ch == 1 and pr > 0:
                    emit_qreduce(pr - 1)

            # ---- k path (DVE muls only need xt; Pool+DVE tree)
            pk = kp.tile([128, NK, 2, D], F16, tag="pk", name=f"pk_{pr}")
            nc.vector.tensor_mul(pk[:, :, 0, :], xt[:, 0:NK, 0:D], kt_t[:])
            nc.vector.tensor_mul(pk[:, :, 1, :], xt[:, 0:NK, D:128], kt_t[:])
            t1 = tp.tile([128, NK, 2, 32], F16, tag="t1", name=f"t1_{pr}")
            nc.gpsimd.tensor_add(t1[:], pk[:, :, :, 0:32], pk[:, :, :, 32:64])
            t2 = tp.tile([128, NK, 2, 16], F16, tag="t2", name=f"t2_{pr}")
            nc.vector.tensor_add(t2[:], t1[:, :, :, 0:16], t1[:, :, :, 16:32])
            t3 = tp.tile([128, NK, 2, 8], F16, tag="t3", name=f"t3_{pr}")
            nc.vector.tensor_add(t3[:], t2[:, :, :, 0:8], t2[:, :, :, 8:16])
            t4 = tp.tile([128, NK, 2, 4], F16, tag="t4", name=f"t4_{pr}")
            nc.gpsimd.tensor_add(t4[:], t3[:, :, :, 0:4], t3[:, :, :, 4:8])
            t5 = tp.tile([128, NK, 2, 2], F16, tag="t5", name=f"t5_{pr}")
            nc.gpsimd.tensor_add(t5[:], t4[:, :, :, 0:2], t4[:, :, :, 2:4])
            ko_s = ok.tile([128, NK, 2], F16, tag="ko", name=f"ko_{pr}")
            nc.gpsimd.tensor_add(ko_s[:], t5[:, :, :, 0], t5[:, :, :, 1])
            nc.sync.dma_start(ko_d[pr], ko_s[:])

            # ---- q products (after drains)
            pq = qp.tile([128, NT, 128], F16, tag="pq", name=f"pq_{pr}")
            nc.vector.tensor_mul(pq[:], peq[:], qt_t[:])
            pq_tiles[pr] = pq

        emit_qreduce(PAIRS - 1)

    nc.compile()
    return nc


def _get_module():
    if "nc" not in _CACHE:
        _CACHE["nc"] = build_module()
    return _CACHE["nc"]


def make_in_maps(queries, keys, noise, conv_weight, num):
    """Host-side shard + re-layout (all cheap numpy ops)."""
    num = int(np.asarray(num))
    queries = np.asarray(queries, dtype=np.float32)
    keys = np.asarray(keys, dtype=np.float32)
    noise = np.asarray(noise, dtype=np.float32)
    w = np.asarray(conv_weight, dtype=np.float32)[0, 0, :]
    scale = 1.0 / math.sqrt(num * D)

    # Toeplitz weights (scale folded in): W_s[p, m] = w[p + 128s - m] * scale
    p = np.arange(128)[:, None]
    m = np.arange(128)[None, :]
    Wq = np.zeros((3, 128, 128), np.float32)
    for s in range(3):
        j = p + 128 * s - m
        mask = (j >= 0) & (j < K)
        Wq[s][mask] = w[j[mask]] * scale
    Wq16 = Wq.astype(np.float16)

    # row selector: sel[rd, r'] = (rd // 64 == r')
    sel = np.zeros((128, 2), np.float16)
    sel[0:D, 0] = 1.0
    sel[D:128, 1] = 1.0

    # xf[c][pair][p, n, (r,d)] = noise[16c + 2*pair + r, d, 128n + p]
    xf = (
        noise[:, :, : NW * 128]
        .reshape(N_CORES, PAIRS, 2, D, NW, 128)
        .transpose(0, 1, 5, 4, 2, 3)
        .reshape(N_CORES, PAIRS, 128, NW, 128)
        .astype(np.float16)
    )
    # qt[b][(r,d), tau, t'] = queries[b, d, 128 tau + t']  (replicated over r)
    qt_half = queries.reshape(B, D, NT, 128).astype(np.float16)
    qt = np.concatenate([qt_half, qt_half], axis=1)  # (B, 128, NT, 128)
    # kt[b][p, n, d] = keys[b, d, 128n + p - 100] * scale (zero out of range)
    kpad = np.zeros((B, D, NK * 128), np.float32)
    kpad[:, :, K // 2 : K // 2 + L] = keys * scale
    kt = kpad.reshape(B, D, NK, 128).transpose(0, 3, 2, 1).astype(np.float16)

    in_maps = []
    for c in range(N_CORES):
        b = c // 2
        in_maps.append(
            {
                "xf": np.ascontiguousarray(xf[c]),
                "wq": Wq16,
                "qt": np.ascontiguousarray(qt[b]),
                "kt": np.ascontiguousarray(kt[b]),
                "sel": sel,
            }
        )
    return in_maps


def assemble_outputs(results):
    qhat = np.empty((B * NUM, L), np.float32)
    khat = np.empty((B * NUM, L), np.float32)
    for c in range(N_CORES):
        qo = results[c]["qo"]  # (PAIRS, 128, NT, 2) f32, [pr, t', tau, r]
        ko = results[c]["ko"].astype(np.float32)  # (PAIRS, 128, NK, 2)
        qarr = qo.transpose(0, 3, 2, 1).reshape(16, NT * 128)
        karr = ko.transpose(0, 3, 2, 1).reshape(16, NK * 128)
        qhat[16 * c : 16 * (c + 1)] = qarr
        khat[16 * c : 16 * (c + 1)] = karr[:, K // 2 : K // 2 + L]
    return (
        qhat.reshape(B, NUM, L),
        khat.reshape(B, NUM, L),
    )


def kernel(queries, keys, noise, conv_weight, num):
    _ensure_paths()
    from concourse import bass_utils

    in_maps = make_in_maps(queries, keys, noise, conv_weight, num)
    nc = _get_module()
    res = bass_utils.run_bass_kernel_spmd(nc, in_maps, core_ids=list(range(N_CORES)))
    return assemble_outputs(res.results)


# revision 4
# speedup vs baseline: 1.5211x; 1.1110x over previous
"""Trainium2 Bass kernel for nn_ConvSPE (depthwise-conv SPE + per-channel contraction).

Math (reference): per bn=(b,nu) row and channel d:
    pe_k = noise / sqrt(num*d)                       (b*num, d, s+2k)
    pe_q = depthwise_valid_xcorr(pe_k, w)            k=200 taps, same filter per channel
    qhat[b,nu,t] = sum_d pe_q[bn,d,t]      * q[b,d,t]
    khat[b,nu,t] = sum_d pe_k[bn,d,t+k//2] * k[b,d,t]

Kernel strategy (8 NeuronCores, data-parallel over 128 bn rows; 8 row-PAIRS/core):
  * Transposed-conv orientation: stationary = x-window [sample, (r,d)],
    moving = Toeplitz W_s[sample, t'] -> PSUM holds pe_q^T [(r,d), t'].
    3 PSUM-accumulated matmuls per (pair, t-block), 128 cols each.
  * q-path: ACT drains PSUM->SBUF fp16; DVE multiplies by replicated q^T
    (fp16 2x); the d-reduction is a PE matmul with the products as the
    STATIONARY and a [128,2] row-selector as moving -> cost 2 cols.
  * k-path: DVE mul vs shifted/scaled keys (fp16 2x); reduce over d via a
    split tree: Pool takes L1 + tail reduce, DVE takes L2+L3.
"""

import math
import numpy as np

_CACHE = {}


def _ensure_paths():
    try:
        import concourse  # noqa: F401
    except ImportError:
        import sys

        for p in ("/opt/trn_rl_repo", "/root/.axon_site/_ro/trn_rl_repo"):
            if p not in sys.path:
                sys.path.insert(0, p)


N_CORES = 8
B, D, L, K, NUM = 4, 64, 4096, 200, 32
NW = 34  # x windows of 128 per pair tile (covers t+j up to 4351)
NT = 32  # output time blocks of 128
NK = 33  # khat product blocks (u = t + 100 spans [0, 4224))
PAIRS = 8  # row-pairs per core (16 rows)


def build_module():
    """Build + compile the per-core Bass module (identical SPMD program)."""
    _ensure_paths()
    from contextlib import ExitStack

    import concourse.bacc as bacc
    import concourse.mybir as mybir
    import concourse.tile as tile

    F16 = mybir.dt.float16
    F32 = mybir.dt.float32
    X = mybir.AxisListType.X

    nc = bacc.Bacc(
        "TRN2", target_bir_lowering=False, debug=False, num_devices=N_CORES
    )

    xf_d = nc.dram_tensor("xf", [PAIRS, 128, NW, 128], F16, kind="ExternalInput").ap()
    wq_d = nc.dram_tensor("wq", [3, 128, 128], F16, kind="ExternalInput").ap()
    qt_d = nc.dram_tensor("qt", [128, NT, 128], F16, kind="ExternalInput").ap()
    kt_d = nc.dram_tensor("kt", [128, NK, D], F16, kind="ExternalInput").ap()
    sel_d = nc.dram_tensor("sel", [128, 2], F16, kind="ExternalInput").ap()
    qo_d = nc.dram_tensor("qo", [PAIRS, 128, NT, 2], F32, kind="ExternalOutput").ap()
    ko_d = nc.dram_tensor("ko", [PAIRS, 128, NK, 2], F16, kind="ExternalOutput").ap()

    with tile.TileContext(nc) as tc, ExitStack() as ctx:
        wp = ctx.enter_context(tc.tile_pool(name="const", bufs=1))
        xp = ctx.enter_context(tc.tile_pool(name="x", bufs=3))
        pp = ctx.enter_context(tc.tile_pool(name="psum", bufs=2, space="PSUM"))
        op_ps = ctx.enter_context(tc.tile_pool(name="opsum", bufs=2, space="PSUM"))
        cp = ctx.enter_context(tc.tile_pool(name="peq", bufs=2))
        qp = ctx.enter_context(tc.tile_pool(name="prodq", bufs=2))
        kp = ctx.enter_context(tc.tile_pool(name="prodk", bufs=2))
        tp = ctx.enter_context(tc.tile_pool(name="tree", bufs=2))
        oq = ctx.enter_context(tc.tile_pool(name="outq", bufs=2))
        ok = ctx.enter_context(tc.tile_pool(name="outk", bufs=2))

        # consts: Toeplitz weights first (small, gates conv start)
        wts = []
        for s in range(3):
            t = wp.tile([128, 128], F16, tag=f"w{s}")
            nc.sync.dma_start(t[:], wq_d[s])
            wts.append(t)
        sel_t = wp.tile([128, 2], F16, tag="sel")
        nc.sync.dma_start(sel_t[:], sel_d[:])

        xts = {}

        def load_x(pr):
            """Two-chunk load so the conv can start on the first half."""
            if pr >= PAIRS:
                return
            xt = xp.tile([128, NW, 128], F16, tag="xt", name=f"xt_{pr}")
            nc.sync.dma_start(xt[:, 0:18, :], xf_d[pr, :, 0:18, :])
            nc.sync.dma_start(xt[:, 18:NW, :], xf_d[pr, :, 18:NW, :])
            xts[pr] = xt

        # first x tile before the bigger consts so conv starts early
        load_x(0)
        kt_t = wp.tile([128, NK, D], F16, tag="kt")
        nc.sync.dma_start(kt_t[:], kt_d[:])
        load_x(1)
        qt_t = wp.tile([128, NT, 128], F16, tag="qt")
        nc.sync.dma_start(qt_t[:], qt_d[:])

        peq_t, pq_t, pk_t, t1_t, t3_t = {}, {}, {}, {}, {}

        def conv_chunk(pr, ch):
            if not (0 <= pr < PAIRS) or pr not in xts:
                return
            xt = xts[pr]
            if ch == 0:
                peq_t[pr] = cp.tile([128, NT, 128], F16, tag="peq", name=f"peq_{pr}")
            ps = pp.tile([128, 8, 128], F32, tag="ps", name=f"ps_{pr}_{ch}")
            for t8 in range(8):
                tau = ch * 8 + t8
                for s in range(3):
                    nc.tensor.matmul(
                        ps[:, t8, :],
                        xt[:, tau + s, :],
                        wts[s][:],
                        start=(s == 0),
                        stop=(s == 2),
                    )
            nc.scalar.copy(peq_t[pr][:, ch * 8 : (ch + 1) * 8, :], ps[:])

        def kmul(pr):
            if not (0 <= pr < PAIRS):
                return
            xt = xts[pr]
            pk = kp.tile([128, NK, 2, D], F16, tag="pk", name=f"pk_{pr}")
            nc.vector.tensor_mul(pk[:, :, 0, :], xt[:, 0:NK, 0:D], kt_t[:])
            nc.vector.tensor_mul(pk[:, :, 1, :], xt[:, 0:NK, D:128], kt_t[:])
            pk_t[pr] = pk

        def qmul(pr):
            if not (0 <= pr < PAIRS):
                return
            pq = qp.tile([128, NT, 128], F16, tag="pq", name=f"pq_{pr}")
            nc.vector.tensor_mul(pq[:], peq_t.pop(pr)[:], qt_t[:])
            pq_t[pr] = pq

        def qreduce(pr):
            """PE d-reduce for pair pr, then drain + DMA out."""
            if not (0 <= pr < PAIRS):
                return
            pq = pq_t.pop(pr)
            po = op_ps.tile([128, NT, 2], F32, tag="po", name=f"po_{pr}")
            for t in range(NT):
                nc.tensor.matmul(
                    po[:, t, :], pq[:, t, :], sel_t[:], start=True, stop=True
                )
            qo_s = oq.tile([128, NT, 2], F32, tag="qo", name=f"qo_{pr}")
            nc.scalar.copy(qo_s[:], po[:])
            nc.sync.dma_start(qo_d[pr], qo_s[:])

        def tree_hi(pr, on_dve=False):
            """L1 (Pool unless on_dve) producing t1."""
            if not (0 <= pr < PAIRS):
                return
            pk = pk_t.pop(pr)
            eng = nc.vector if on_dve else nc.gpsimd
            t1 = tp.tile([128, NK, 2, 32], F16, tag="t1", name=f"t1_{pr}")
            eng.tensor_add(t1[:], pk[:, :, :, 0:32], pk[:, :, :, 32:64])
            t1_t[pr] = t1

        def tree_mid(pr):
            """L2+L3 on DVE producing t3."""
            if not (0 <= pr < PAIRS):
                return
            t1 = t1_t.pop(pr)
            t2 = tp.tile([128, NK, 2, 16], F16, tag="t2", name=f"t2_{pr}")
            nc.vector.tensor_add(t2[:], t1[:, :, :, 0:16], t1[:, :, :, 16:32])
            t3 = tp.tile([128, NK, 2, 8], F16, tag="t3", name=f"t3_{pr}")
            nc.vector.tensor_add(t3[:], t2[:, :, :, 0:8], t2[:, :, :, 8:16])
            t3_t[pr] = t3

        def tree_tail(pr, on_dve=False):
            """L4-L6 (Pool unless on_dve) -> ko DMA."""
            if not (0 <= pr < PAIRS):
                return
            t3 = t3_t.pop(pr)
            eng = nc.vector if on_dve else nc.gpsimd
            t4 = tp.tile([128, NK, 2, 4], F16, tag="t4", name=f"t4_{pr}")
            eng.tensor_add(t4[:], t3[:, :, :, 0:4], t3[:, :, :, 4:8])
            t5 = tp.tile([128, NK, 2, 2], F16, tag="t5", name=f"t5_{pr}")
            eng.tensor_add(t5[:], t4[:, :, :, 0:2], t4[:, :, :, 2:4])
            ko_s = ok.tile([128, NK, 2], F16, tag="ko", name=f"ko_{pr}")
            eng.tensor_add(ko_s[:], t5[:, :, :, 0], t5[:, :, :, 1])
            nc.sync.dma_start(ko_d[pr], ko_s[:])

        LAST = PAIRS - 1
        for i in range(PAIRS + 3):
            load_x(i + 2)
            kmul(i)
            conv_chunk(i, 0)
            conv_chunk(i, 1)
            qreduce(i - 2)
            conv_chunk(i, 2)
            conv_chunk(i, 3)
            qmul(i - 1)
            # last pair's tree entirely on DVE, early, to shorten the tail
            if i == LAST:
                tree_hi(LAST, on_dve=True)
                tree_mid(LAST)
                tree_tail(LAST, on_dve=True)
            if i - 1 != LAST:
                tree_hi(i - 1)
            if i - 2 != LAST:
                tree_mid(i - 2)
            if i - 3 != LAST:
                tree_tail(i - 3)

    nc.compile()
    return nc


def _get_module():
    if "nc" not in _CACHE:
        _CACHE["nc"] = build_module()
    return _CACHE["nc"]


def make_in_maps(queries, keys, noise, conv_weight, num):
    """Host-side shard + re-layout (all cheap numpy ops)."""
    num = int(np.asarray(num))
    queries = np.asarray(queries, dtype=np.float32)
    keys = np.asarray(keys, dtype=np.float32)
    noise = np.asarray(noise, dtype=np.float32)
    w = np.asarray(conv_weight, dtype=np.float32)[0, 0, :]
    scale = 1.0 / math.sqrt(num * D)

    # Toeplitz weights (scale folded in): W_s[p, m] = w[p + 128s - m] * scale
    p = np.arange(128)[:, None]
    m = np.arange(128)[None, :]
    Wq = np.zeros((3, 128, 128), np.float32)
    for s in range(3):
        j = p + 128 * s - m
        mask = (j >= 0) & (j < K)
        Wq[s][mask] = w[j[mask]] * scale
    Wq16 = Wq.astype(np.float16)

    # row selector: sel[rd, r'] = (rd // 64 == r')
    sel = np.zeros((128, 2), np.float16)
    sel[0:D, 0] = 1.0
    sel[D:128, 1] = 1.0

    # xf[c][pair][p, n, (r,d)] = noise[16c + 2*pair + r, d, 128n + p]
    xf = (
        noise[:, :, : NW * 128]
        .reshape(N_CORES, PAIRS, 2, D, NW, 128)
        .transpose(0, 1, 5, 4, 2, 3)
        .reshape(N_CORES, PAIRS, 128, NW, 128)
        .astype(np.float16)
    )
    # qt[b][(r,d), tau, t'] = queries[b, d, 128 tau + t']  (replicated over r)
    qt_half = queries.reshape(B, D, NT, 128).astype(np.float16)
    qt = np.concatenate([qt_half, qt_half], axis=1)  # (B, 128, NT, 128)
    # kt[b][p, n, d] = keys[b, d, 128n + p - 100] * scale (zero out of range)
    kpad = np.zeros((B, D, NK * 128), np.float32)
    kpad[:, :, K // 2 : K // 2 + L] = keys * scale
    kt = kpad.reshape(B, D, NK, 128).transpose(0, 3, 2, 1).astype(np.float16)

    in_maps = []
    for c in range(N_CORES):
        b = c // 2
        in_maps.append(
            {
                "xf": np.ascontiguousarray(xf[c]),
                "wq": Wq16,
                "qt": np.ascontiguousarray(qt[b]),
                "kt": np.ascontiguousarray(kt[b]),
                "sel": sel,
            }
        )
    return in_maps


def assemble_outputs(results):
    qhat = np.empty((B * NUM, L), np.float32)
    khat = np.empty((B * NUM, L), np.float32)
    for c in range(N_CORES):
        qo = results[c]["qo"]  # (PAIRS, 128, NT, 2) f32, [pr, t', tau, r]
        ko = results[c]["ko"].astype(np.float32)  # (PAIRS, 128, NK, 2)
        qarr = qo.transpose(0, 3, 2, 1).reshape(16, NT * 128)
        karr = ko.transpose(0, 3, 2, 1).reshape(16, NK * 128)
        qhat[16 * c : 16 * (c + 1)] = qarr
        khat[16 * c : 16 * (c + 1)] = karr[:, K // 2 : K // 2 + L]
    return (
        qhat.reshape(B, NUM, L),
        khat.reshape(B, NUM, L),
    )


def kernel(queries, keys, noise, conv_weight, num):
    _ensure_paths()
    from concourse import bass_utils

    in_maps = make_in_maps(queries, keys, noise, conv_weight, num)
    nc = _get_module()
    res = bass_utils.run_bass_kernel_spmd(nc, in_maps, core_ids=list(range(N_CORES)))
    return assemble_outputs(res.results)


# revision 11
# speedup vs baseline: 1.6576x; 1.0897x over previous
"""Trainium2 Bass kernel for nn_ConvSPE (depthwise-conv SPE + per-channel contraction).

Math (reference): per bn=(b,nu) row and channel d:
    pe_k = noise / sqrt(num*d)                       (b*num, d, s+2k)
    pe_q = depthwise_valid_xcorr(pe_k, w)            k=200 taps, same filter per channel
    qhat[b,nu,t] = sum_d pe_q[bn,d,t]      * q[b,d,t]
    khat[b,nu,t] = sum_d pe_k[bn,d,t+k//2] * k[b,d,t]

Kernel strategy (8 NeuronCores, data-parallel over 128 bn rows; 8 row-PAIRS/core):
  * Transposed-conv orientation: stationary = x-window [sample, (r,d)],
    moving = Toeplitz W_s[sample, t'] -> PSUM holds pe_q^T [(r,d), t'].
    3 PSUM-accumulated matmuls per (pair, t-block), 128 cols each.
  * q-path: ACT drains PSUM->SBUF fp16; DVE multiplies by replicated q^T
    (fp16 2x); the d-reduction is a PE matmul with the products as the
    STATIONARY and a [128,2] row-selector as moving -> cost 2 cols.
  * k-path: DVE mul vs shifted/scaled keys (fp16 2x); reduce over d via a
    split tree: Pool takes L1 + tail reduce, DVE takes L2+L3.
"""

import math
import numpy as np

_CACHE = {}


def _ensure_paths():
    try:
        import concourse  # noqa: F401
    except ImportError:
        import sys

        for p in ("/opt/trn_rl_repo", "/root/.axon_site/_ro/trn_rl_repo"):
            if p not in sys.path:
                sys.path.insert(0, p)


N_CORES = 8
B, D, L, K, NUM = 4, 64, 4096, 200, 32
NW = 34  # x windows of 128 per pair tile (covers t+j up to 4351)
NT = 32  # output time blocks of 128
NK = 33  # khat product blocks (u = t + 100 spans [0, 4224))
PAIRS = 8  # row-pairs per core (16 rows)


def build_module():
    """Build + compile the per-core Bass module (identical SPMD program)."""
    _ensure_paths()
    from contextlib import ExitStack

    import concourse.bacc as bacc
    import concourse.mybir as mybir
    import concourse.tile as tile

    F16 = mybir.dt.float16
    F32 = mybir.dt.float32
    X = mybir.AxisListType.X

    nc = bacc.Bacc(
        "TRN2", target_bir_lowering=False, debug=False, num_devices=N_CORES
    )

    xf_d = nc.dram_tensor("xf", [PAIRS, 128, NW, 128], F16, kind="ExternalInput").ap()
    # packed consts: [3*128 Toeplitz cols | 2 selector cols]
    wq_d = nc.dram_tensor("wq", [128, 3 * 128 + 2], F16, kind="ExternalInput").ap()
    qt_d = nc.dram_tensor("qt", [128, NT, 128], F16, kind="ExternalInput").ap()
    kt_d = nc.dram_tensor("kt", [128, NK, D], F16, kind="ExternalInput").ap()
    qo_d = nc.dram_tensor("qo", [PAIRS, 128, NT, 2], F32, kind="ExternalOutput").ap()
    ko_d = nc.dram_tensor("ko", [PAIRS, 128, NK, 2], F16, kind="ExternalOutput").ap()

    with tile.TileContext(nc) as tc, ExitStack() as ctx:
        wp = ctx.enter_context(tc.tile_pool(name="const", bufs=1))
        xp = ctx.enter_context(tc.tile_pool(name="x", bufs=3))
        pp = ctx.enter_context(tc.tile_pool(name="psum", bufs=2, space="PSUM"))
        op_ps = ctx.enter_context(tc.tile_pool(name="opsum", bufs=2, space="PSUM"))
        cp = ctx.enter_context(tc.tile_pool(name="peq", bufs=2))
        qp = ctx.enter_context(tc.tile_pool(name="prodq", bufs=2))
        kp = ctx.enter_context(tc.tile_pool(name="prodk", bufs=2))
        tp = ctx.enter_context(tc.tile_pool(name="tree", bufs=2))
        oq = ctx.enter_context(tc.tile_pool(name="outq", bufs=2))
        ok = ctx.enter_context(tc.tile_pool(name="outk", bufs=2))

        # consts: Toeplitz weights + selector in one DMA (small, gates conv)
        wq_t = wp.tile([128, 3 * 128 + 2], F16, tag="wq")
        nc.sync.dma_start(wq_t[:], wq_d[:])
        wts = [wq_t[:, 128 * s : 128 * (s + 1)] for s in range(3)]
        sel_t = wq_t[:, 384:386]

        xts = {}

        def load_x(pr):
            """Two-chunk load so the conv can start on the first half."""
            if pr >= PAIRS:
                return
            xt = xp.tile([128, NW, 128], F16, tag="xt", name=f"xt_{pr}")
            nc.sync.dma_start(xt[:, 0:18, :], xf_d[pr, :, 0:18, :])
            nc.sync.dma_start(xt[:, 18:NW, :], xf_d[pr, :, 18:NW, :])
            xts[pr] = xt

        # first x tile before the bigger consts so conv starts early
        load_x(0)
        kt_t = wp.tile([128, NK, D], F16, tag="kt")
        nc.sync.dma_start(kt_t[:], kt_d[:])
        load_x(1)
        qt_t = wp.tile([128, NT, 128], F16, tag="qt")
        nc.sync.dma_start(qt_t[:], qt_d[:])

        peq_t, pq_t, pk_t, t1_t, t3_t = {}, {}, {}, {}, {}

        def conv_chunk(pr, ch):
            if not (0 <= pr < PAIRS) or pr not in xts:
                return
            xt = xts[pr]
            if ch == 0:
                peq_t[pr] = cp.tile([128, NT, 128], F16, tag="peq", name=f"peq_{pr}")
            ps = pp.tile([128, 8, 128], F32, tag="ps", name=f"ps_{pr}_{ch}")
            for t8 in range(8):
                tau = ch * 8 + t8
                # W_2 is zero for output cols < 57: emit it narrow, mid-group
                nc.tensor.matmul(
                    ps[:, t8, :], xt[:, tau, :], wts[0], start=True, stop=False
                )
                nc.tensor.matmul(
                    ps[:, t8, 57:128],
                    xt[:, tau + 2, :],
                    wq_t[:, 313:384],
                    start=False,
                    stop=False,
                    skip_group_check=True,
                )
                nc.tensor.matmul(
                    ps[:, t8, :], xt[:, tau + 1, :], wts[1], start=False, stop=True
                )
            nc.scalar.copy(peq_t[pr][:, ch * 8 : (ch + 1) * 8, :], ps[:])

        def kmul(pr):
            if not (0 <= pr < PAIRS):
                return
            xt = xts[pr]
            pk = kp.tile([128, NK, 2, D], F16, tag="pk", name=f"pk_{pr}")
            nc.vector.tensor_mul(pk[:, :, 0, :], xt[:, 0:NK, 0:D], kt_t[:])
            nc.vector.tensor_mul(pk[:, :, 1, :], xt[:, 0:NK, D:128], kt_t[:])
            pk_t[pr] = pk

        def qmul(pr):
            if not (0 <= pr < PAIRS):
                return
            pq = qp.tile([128, NT, 128], F16, tag="pq", name=f"pq_{pr}")
            nc.vector.tensor_mul(pq[:], peq_t.pop(pr)[:], qt_t[:])
            pq_t[pr] = pq

        def qreduce(pr):
            """PE d-reduce for pair pr, then drain + DMA out."""
            if not (0 <= pr < PAIRS):
                return
            pq = pq_t.pop(pr)
            po = op_ps.tile([128, NT, 2], F32, tag="po", name=f"po_{pr}")
            for t in range(NT):
                nc.tensor.matmul(
                    po[:, t, :], pq[:, t, :], sel_t, start=True, stop=True
                )
            qo_s = oq.tile([128, NT, 2], F32, tag="qo", name=f"qo_{pr}")
            nc.scalar.copy(qo_s[:], po[:])
            nc.sync.dma_start(qo_d[pr], qo_s[:])

        def tree_hi(pr, on_dve=False):
            """L1 (Pool unless on_dve) producing t1."""
            if not (0 <= pr < PAIRS):
                return
            pk = pk_t.pop(pr)
            eng = nc.vector if on_dve else nc.gpsimd
            t1 = tp.tile([128, NK, 2, 32], F16, tag="t1", name=f"t1_{pr}")
            eng.tensor_add(t1[:], pk[:, :, :, 0:32], pk[:, :, :, 32:64])
            t1_t[pr] = t1

        def tree_mid(pr):
            """L2+L3 on DVE producing t3."""
            if not (0 <= pr < PAIRS):
                return
            t1 = t1_t.pop(pr)
            t2 = tp.tile([128, NK, 2, 16], F16, tag="t2", name=f"t2_{pr}")
            nc.vector.tensor_add(t2[:], t1[:, :, :, 0:16], t1[:, :, :, 16:32])
            t3 = tp.tile([128, NK, 2, 8], F16, tag="t3", name=f"t3_{pr}")
            nc.vector.tensor_add(t3[:], t2[:, :, :, 0:8], t2[:, :, :, 8:16])
            t3_t[pr] = t3

        def tree_tail(pr, on_dve=False):
            """L4-L6 (Pool unless on_dve) -> ko DMA."""
            if not (0 <= pr < PAIRS):
                return
            t3 = t3_t.pop(pr)
            eng = nc.vector if on_dve else nc.gpsimd
            t4 = tp.tile([128, NK, 2, 4], F16, tag="t4", name=f"t4_{pr}")
            eng.tensor_add(t4[:], t3[:, :, :, 0:4], t3[:, :, :, 4:8])
            t5 = tp.tile([128, NK, 2, 2], F16, tag="t5", name=f"t5_{pr}")
            eng.tensor_add(t5[:], t4[:, :, :, 0:2], t4[:, :, :, 2:4])
            ko_s = ok.tile([128, NK, 2], F16, tag="ko", name=f"ko_{pr}")
            eng.tensor_add(ko_s[:], t5[:, :, :, 0], t5[:, :, :, 1])
            nc.sync.dma_start(ko_d[pr], ko_s[:])

        LAST = PAIRS - 1
        for i in range(PAIRS + 2):
            load_x(i + 2)
            kmul(i)
            conv_chunk(i, 0)
            conv_chunk(i, 1)
            qreduce(i - 2)
            conv_chunk(i, 2)
            conv_chunk(i, 3)
            qmul(i - 1)
            if i - 1 == LAST:
                # last pair's tree entirely on DVE to shorten the tail
                tree_hi(LAST, on_dve=True)
                tree_mid(LAST)
                tree_tail(LAST, on_dve=True)
            else:
                tree_hi(i - 1)
                tree_mid(i - 1)
            if i - 2 != LAST:
                tree_tail(i - 2)

    nc.compile()
    return nc


def _get_module():
    if "nc" not in _CACHE:
        _CACHE["nc"] = build_module()
    return _CACHE["nc"]


def make_in_maps(queries, keys, noise, conv_weight, num):
    """Host-side shard + re-layout (all cheap numpy ops)."""
    num = int(np.asarray(num))
    queries = np.asarray(queries, dtype=np.float32)
    keys = np.asarray(keys, dtype=np.float32)
    noise = np.asarray(noise, dtype=np.float32)
    w = np.asarray(conv_weight, dtype=np.float32)[0, 0, :]
    scale = 1.0 / math.sqrt(num * D)

    # Toeplitz weights (scale folded in): W_s[p, m] = w[p + 128s - m] * scale
    p = np.arange(128)[:, None]
    m = np.arange(128)[None, :]
    Wq = np.zeros((3, 128, 128), np.float32)
    for s in range(3):
        j = p + 128 * s - m
        mask = (j >= 0) & (j < K)
        Wq[s][mask] = w[j[mask]] * scale
    Wq16 = Wq.astype(np.float16)

    # row selector: sel[rd, r'] = (rd // 64 == r')
    sel = np.zeros((128, 2), np.float16)
    sel[0:D, 0] = 1.0
    sel[D:128, 1] = 1.0
    # packed [128, 3*128+2]: Toeplitz cols s-major then selector
    wq_pack = np.concatenate(
        [Wq16.transpose(1, 0, 2).reshape(128, 3 * 128), sel], axis=1
    )

    # xf[c][pair][p, n, (r,d)] = noise[16c + 2*pair + r, d, 128n + p]
    xf = (
        noise[:, :, : NW * 128]
        .reshape(N_CORES, PAIRS, 2, D, NW, 128)
        .transpose(0, 1, 5, 4, 2, 3)
        .reshape(N_CORES, PAIRS, 128, NW, 128)
        .astype(np.float16)
    )
    # qt[b][(r,d), tau, t'] = queries[b, d, 128 tau + t']  (replicated over r)
    qt_half = queries.reshape(B, D, NT, 128).astype(np.float16)
    qt = np.concatenate([qt_half, qt_half], axis=1)  # (B, 128, NT, 128)
    # kt[b][p, n, d] = keys[b, d, 128n + p - 100] * scale (zero out of range)
    kpad = np.zeros((B, D, NK * 128), np.float32)
    kpad[:, :, K // 2 : K // 2 + L] = keys * scale
    kt = kpad.reshape(B, D, NK, 128).transpose(0, 3, 2, 1).astype(np.float16)

    in_maps = []
    for c in range(N_CORES):
        b = c // 2
        in_maps.append(
            {
                "xf": np.ascontiguousarray(xf[c]),
                "wq": wq_pack,
                "qt": np.ascontiguousarray(qt[b]),
                "kt": np.ascontiguousarray(kt[b]),
            }
        )
    return in_maps


def assemble_outputs(results):
    qhat = np.empty((B * NUM, L), np.float32)
    khat = np.empty((B * NUM, L), np.float32)
    for c in range(N_CORES):
        qo = results[c]["qo"]  # (PAIRS, 128, NT, 2) f32, [pr, t', tau, r]
        ko = results[c]["ko"].astype(np.float32)  # (PAIRS, 128, NK, 2)
        qarr = qo.transpose(0, 3, 2, 1).reshape(16, NT * 128)
        karr = ko.transpose(0, 3, 2, 1).reshape(16, NK * 128)
        qhat[16 * c : 16 * (c + 1)] = qarr
        khat[16 * c : 16 * (c + 1)] = karr[:, K // 2 : K // 2 + L]
    return (
        qhat.reshape(B, NUM, L),
        khat.reshape(B, NUM, L),
    )


def kernel(queries, keys, noise, conv_weight, num):
    _ensure_paths()
    from concourse import bass_utils

    in_maps = make_in_maps(queries, keys, noise, conv_weight, num)
    nc = _get_module()
    res = bass_utils.run_bass_kernel_spmd(nc, in_maps, core_ids=list(range(N_CORES)))
    return assemble_outputs(res.results)
